# revision 35
# baseline (speedup 1.0000x reference)
"""ConformerAttention (Transformer-XL relative attention) on 8 TRN2 NeuronCores.

Sharding: batch*heads across cores. Core c handles batch b = c//4 and the head
pair (h0, h1) = (2*(c%4), 2*(c%4)+1). All projections, the rel-shift bias, the
softmax and the attention are computed per (b, head-pair) on one core; the
output projection is computed per-core against that pair's W_out columns and
the 4 partial [T, D] outputs per batch are summed on the host (the gather).

v2 dataflow (transposed scores, PE-side bias merge):
  - Scores are computed directly TRANSPOSED: qkT[s, q] = (K^T)^T_chunk @ Qu
    per (s-128-chunk, q-512-tile), FD=512, K=64.  The two heads' matmuls are
    emitted back-to-back on row groups (0,0)/(64,0) into the two banks of one
    [128, 1024] PSUM tile, so they run CONCURRENTLY on the PE array halves.
  - The rel-shift bias band bd_al[q, s] (built per q-128-tile via the skewed
    strip + SBUF->SBUF DMA, as before) is merged by a plain matmul
    out += bd_al^T = lhsT(bd_al) @ I  with start=False, accumulating straight
    into the qkT PSUM bank.  No DVE add, no score staging, no extra casts.
  - One exp() per s-chunk drains the combined [128, 1024] PSUM (both heads)
    into probsT[s, (h,q)] bf16, which feeds attn@[V|1] directly (row sum Z
    rides as the 65th rhs column, as before).
  - av is emitted one s-chunk behind exp so the PE never waits on ACT.
"""

import os

import numpy as np

T = 2048
D = 512
NH = 8
DK = 64
P = 2 * T - 1
NCORES = 8
NQT = T // 128  # 16 q-subtiles of 128 rows
NSC = T // 128  # 16 s-chunks of 128 cols
SW = T + 127  # 2175 band strip width per q-subtile
SCALE = np.float32(1.0 / np.sqrt(DK))

_NC = None
_LAST_RESULTS = None


def _dt(name, default):
    import concourse.mybir as mybir

    return {"f32": mybir.dt.float32, "bf16": mybir.dt.bfloat16}[
        os.environ.get(name, default)
    ]


def _dtypes():
    return (
        _dt("KERNEL_PROJDT", "bf16"),
        _dt("KERNEL_MMDT", "bf16"),
        _dt("KERNEL_BANDDT", "bf16"),
        _dt("KERNEL_SDT", "bf16"),
    )


def _np_dt(dt):
    import concourse.mybir as mybir

    return mybir.dt.np(dt)


def _build_nc():
    import concourse.bacc as bacc
    import concourse.bass as bass
    import concourse.mybir as mybir
    import concourse.tile as tile
    from concourse import masks

    F32 = mybir.dt.float32
    PROJDT, MMDT, BANDDT, SDT = _dtypes()
    AF = mybir.ActivationFunctionType

    nc = bacc.Bacc("TRN2", target_bir_lowering=False, debug=False)

    xT_d = nc.dram_tensor("xT", [D, T], PROJDT, kind="ExternalInput")
    posTe_d = nc.dram_tensor("posTe", [D, P], PROJDT, kind="ExternalInput")
    wqT_d = nc.dram_tensor("wqT", [128, 512], PROJDT, kind="ExternalInput")
    wkT_d = nc.dram_tensor("wkT", [128, 512], PROJDT, kind="ExternalInput")
    wvT_d = nc.dram_tensor("wvT", [128, 512], PROJDT, kind="ExternalInput")
    wposT_d = nc.dram_tensor("wposT", [128, 512], PROJDT, kind="ExternalInput")
    woT_d = nc.dram_tensor("woT", [128, D], F32, kind="ExternalInput")
    bu_d = nc.dram_tensor("bias_u", [128, 1], F32, kind="ExternalInput")
    bv_d = nc.dram_tensor("bias_v", [128, 1], F32, kind="ExternalInput")
    out_d = nc.dram_tensor("outp", [T, D], F32, kind="ExternalOutput")

    with tile.TileContext(nc) as tc:
        with (
            tc.tile_pool(name="const", bufs=1) as constp,
            tc.tile_pool(name="pers", bufs=1) as pers,
            # PSUM budget (8 banks): qkT 2x[128,1024] = 4, sps 2x[128,512] = 2,
            # av 2x[128,260] = 2
            tc.tile_pool(name="qkps", bufs=2, space="PSUM") as qkp,
            tc.tile_pool(name="sps", bufs=2, space="PSUM") as sps,
            tc.tile_pool(name="avps", bufs=1, space="PSUM") as avp,
            tc.tile_pool(name="sb1", bufs=2) as sb1,
        ):
            ident_f32 = constp.tile([128, 128], F32)
            masks.make_identity(nc, ident_f32[:])
            ident_b = constp.tile([128, 128], BANDDT)
            masks.make_identity(nc, ident_b[:])

            bu_sb = constp.tile([128, 1], F32)
            nc.gpsimd.dma_start(out=bu_sb[:], in_=bu_d.ap())
            bv_sb = constp.tile([128, 1], F32)
            nc.gpsimd.dma_start(out=bv_sb[:], in_=bv_d.ap())
            woT_sb = constp.tile([128, D], F32)
            nc.gpsimd.dma_start(out=woT_sb[:], in_=woT_d.ap())

            QuT = pers.tile([128, T], MMDT)
            QvT = pers.tile([128, T], MMDT)
            KT = pers.tile([128, T], MMDT)
            posT = pers.tile([128, P], MMDT)
            Vsb = pers.tile([128, NSC * 130], SDT)
            aoT = pers.tile([128, T], F32)

            # ones columns for the fused row-sum (col 64 of each rhs slice)
            v3 = Vsb[:].rearrange("p (j c) -> p j c", c=130)
            nc.vector.memset(v3[:, :, 64:65], 1.0)
            nc.vector.memset(v3[:, :, 129:130], 1.0)

            # ---------------- phase 0: projections ----------------
            with (
                tc.tile_pool(name="posp", bufs=1) as posp,
                tc.tile_pool(name="ph0", bufs=1) as ph0p,
            ):
                xT_sb = []
                for kc in range(4):
                    t = ph0p.tile([128, T], PROJDT, tag=f"xT{kc}")
                    nc.sync.dma_start(
                        out=t[:], in_=xT_d.ap()[128 * kc : 128 * (kc + 1), :]
                    )
                    xT_sb.append(t)

                def load_wT(dr, pool, nm):
                    t = pool.tile([128, 512], PROJDT, tag=nm, name=nm)
                    nc.sync.dma_start(out=t[:], in_=dr.ap())
                    return [t[:, 128 * kc : 128 * (kc + 1)] for kc in range(4)]

                wq_sb = load_wT(wqT_d, ph0p, "wq")
                wk_sb = load_wT(wkT_d, ph0p, "wk")
                wv_sb = load_wT(wvT_d, ph0p, "wv")
                wp_sb = load_wT(wposT_d, posp, "wp")

                pos_stage = {}

                def load_pos_chunk(n8):
                    w0 = 512 * n8
                    ncols = min(512, P - w0)
                    ts = []
                    for kc in range(4):
                        t = posp.tile(
                            [128, 512], PROJDT, tag=f"pe{kc}", bufs=3,
                            name=f"pe{kc}_{n8}",
                        )
                        nc.gpsimd.dma_start(
                            out=t[:, :ncols],
                            in_=posTe_d.ap()[
                                128 * kc : 128 * (kc + 1), w0 : w0 + ncols
                            ],
                        )
                        ts.append(t)
                    pos_stage[n8] = ts

                def mm_pos_chunk(n8):
                    w0 = 512 * n8
                    ncols = min(512, P - w0)
                    pe_sb = pos_stage.pop(n8)
                    ps = sps.tile([128, 512], F32, tag="sp", name=f"p0p_{n8}")
                    for kc in range(4):
                        nc.tensor.matmul(
                            ps[:, :ncols],
                            wp_sb[kc][:],
                            pe_sb[kc][:, :ncols],
                            start=(kc == 0),
                            stop=(kc == 3),
                        )
                    nc.vector.tensor_copy(posT[:, w0 : w0 + ncols], ps[:, :ncols])

                # prefetch the first pos chunks; their loads overlap Q/K/V
                for n8 in (7, 6, 5):
                    load_pos_chunk(n8)

                # Q^T and K^T (both heads stacked on partitions)
                for w_sb, drains in (
                    (
                        wq_sb,
                        lambda ps, sl: (
                            nc.scalar.activation(
                                QuT[:, sl], ps[:], AF.Identity, bias=bu_sb[:]
                            ),
                            nc.scalar.activation(
                                QvT[:, sl], ps[:], AF.Identity, bias=bv_sb[:]
                            ),
                        ),
                    ),
                    (
                        wk_sb,
                        lambda ps, sl: nc.vector.tensor_copy(KT[:, sl], ps[:]),
                    ),
                ):
                    for n4 in range(4):
                        sl = slice(512 * n4, 512 * (n4 + 1))
                        ps = sps.tile([128, 512], F32, tag="sp", name=f"p0_{n4}")
                        for kc in range(4):
                            nc.tensor.matmul(
                                ps[:],
                                w_sb[kc][:],
                                xT_sb[kc][:, sl],
                                start=(kc == 0),
                                stop=(kc == 3),
                            )
                        drains(ps, sl)

                # V (both heads)
                for tt in range(NSC):
                    ps = sps.tile([128, 512], F32, tag="sp", name=f"pv_{tt}")
                    for kc in range(4):
                        nc.tensor.matmul(
                            ps[:, 0:128],
                            xT_sb[kc][:, 128 * tt : 128 * (tt + 1)],
                            wv_sb[kc][:],
                            start=(kc == 0),
                            stop=(kc == 3),
                        )
                    nc.vector.tensor_copy(
                        Vsb[:, 130 * tt : 130 * tt + 64], ps[:, 0:64]
                    )
                    nc.vector.tensor_copy(
                        Vsb[:, 130 * tt + 65 : 130 * tt + 129], ps[:, 64:128]
                    )

                # ---------------- phase 1: attention ----------------
                # pos chunks projected lazily: qt512=0 needs chunks 3..7,
                # each later qt512 needs one lower chunk.
                mm_pos_chunk(7)
                load_pos_chunk(4)
                mm_pos_chunk(6)
                load_pos_chunk(3)
                for n8 in (5, 4, 3):
                    mm_pos_chunk(n8)

                strip_cur = {}

                def emit_strip_op(qt5, i, h, sc5, out):
                    """One 512-col band sub-chunk matmul + drain for
                    (q-subtile i, head h) of tile qt5.  After the last
                    sub-chunk, issues the skewed rel-shift DMA into out."""
                    q0 = 512 * qt5 + 128 * i
                    qbar = (T - 1) - q0 - 127
                    w = 512 if sc5 < 4 else SW - 4 * 512
                    po = 64 * h
                    key = (qt5, i, h)
                    if key not in strip_cur:
                        strip_cur[key] = sb1.tile(
                            [128, SW], BANDDT, tag=f"strip{i}_{h}", bufs=2,
                            name=f"strip{i}_{h}_{qt5}",
                        )
                    strip = strip_cur[key]
                    ps_bd = sps.tile(
                        [128, 512], F32, tag="sp",
                        name=f"bd_{qt5}_{i}_{h}_{sc5}",
                    )
                    nc.tensor.matmul(
                        ps_bd[:, :w],
                        QvT[po : po + 64, q0 : q0 + 128],
                        posT[
                            po : po + 64,
                            qbar + 512 * sc5 : qbar + 512 * sc5 + w,
                        ],
                        start=True,
                        stop=True,
                    )
                    # big drains on DVE (ACT's FIFO must stay clear for exp);
                    # only the cheap 127-col tails go to ACT
                    if sc5 == 4:
                        nc.scalar.copy(
                            strip[:, 512 * sc5 : 512 * sc5 + w], ps_bd[:, :w]
                        )
                    else:
                        nc.vector.tensor_copy(
                            strip[:, 512 * sc5 : 512 * sc5 + w], ps_bd[:, :w]
                        )
                    if sc5 == 4:
                        t = sb1.tile(
                            [128, T], BANDDT, tag=f"bdal{i}_{h}", bufs=2,
                            name=f"bdal{i}_{h}_{qt5}",
                        )
                        eng = nc.sync if h == 0 else nc.scalar
                        eng.dma_start(
                            out=t[:],
                            in_=bass.AP(
                                strip.tensor, 127, [[SW - 1, 128], [1, T]]
                            ),
                        )
                        out[(i, h)] = t

                # pipeline fill: strips + skew for tile 0
                bd_al = {}
                for i in range(4):
                    for h in range(2):
                        for sc5 in range(5):
                            emit_strip_op(0, i, h, sc5, bd_al)

                # next-tile band work, spread through the s-chunk loop:
                # per (i, h) the sub-chunks run (1,2,3,0,4) so the ones
                # touching the freshly-projected low pos chunk come later
                STRIP_TASKS = [
                    (i, h, s5)
                    for i in range(4)
                    for s5 in (1, 2, 3, 0, 4)
                    for h in range(2)
                ]

                for qt5 in range(4):
                    q0t = 512 * qt5  # q-512 tile base
                    bd_al_next = {}
                    sti = [0]

                    def pop_strip():
                        if qt5 < 3 and sti[0] < len(STRIP_TASKS):
                            i_, h_, s5_ = STRIP_TASKS[sti[0]]
                            sti[0] += 1
                            emit_strip_op(qt5 + 1, i_, h_, s5_, bd_al_next)

                    # --- s-chunk loop ---
                    # [128, 512] = exactly one PSUM bank per head; av slices at
                    # 65-col offsets share the bank. Only the FIRST matmul into
                    # the bank may use start=True: start marks the whole
                    # 2 KiB zero-region pending, so a second start would wipe
                    # sibling slices' sc=0 contribution.
                    avs = [
                        avp.tile([128, 512], F32, tag=f"av{h}", name=f"av{h}_{qt5}")
                        for h in range(2)
                    ]
                    prev = None  # (probsT tile, sc) pending av
                    for sc in range(NSC):
                        s0 = 128 * sc
                        if sc == 0 and qt5 < 3:
                            load_pos_chunk(2 - qt5)
                        if sc == 1 and qt5 < 3:
                            mm_pos_chunk(2 - qt5)
                        ps = qkp.tile([128, 1024], F32, tag="qkT")
                        # content scores, both heads back-to-back on row
                        # groups (0,0)/(64,0) -> the two banks of ps
                        for h in range(2):
                            po = 64 * h
                            nc.tensor.matmul(
                                ps[:, 512 * h : 512 * h + 512],
                                KT[po : po + 64, s0 : s0 + 128],
                                QuT[po : po + 64, q0t : q0t + 512],
                                start=True,
                                stop=False,
                            )
                        if sc >= 1:
                            pop_strip()
                        # merge rel-shift bias: ps += bd_al^T via identity
                        for h in range(2):
                            for i in range(4):
                                nc.tensor.matmul(
                                    ps[:, 512 * h + 128 * i : 512 * h + 128 * i + 128],
                                    bd_al[(i, h)][:, s0 : s0 + 128],
                                    ident_b[:],
                                    start=False,
                                    stop=(i == 3),
                                )
                            if sc >= 1:
                                pop_strip()
                        probsT = sb1.tile([128, 1024], SDT, tag="probsT", bufs=2)
                        # per-head exp: h0's drain starts while h1's bias
                        # merges are still on the PE
                        for h in range(2):
                            nc.scalar.activation(
                                probsT[:, 512 * h : 512 * h + 512],
                                ps[:, 512 * h : 512 * h + 512],
                                AF.Exp,
                            )

                        if prev is not None:
                            _emit_av(nc, prev[0], prev[1], avs, Vsb)
                        prev = (probsT, sc)
                        if sc >= 1:
                            pop_strip()
                    _emit_av(nc, prev[0], prev[1], avs, Vsb)
                    bd_al = bd_al_next

                    # --- epilogue: normalize, transpose, project ---
                    for i in range(4):
                        q0 = q0t + 128 * i
                        for h in range(2):
                            po = 64 * h
                            av = avs[h]
                            rz = sb1.tile([128, 1], F32, tag="rz")
                            nc.vector.reciprocal(
                                rz[:], av[:, 65 * i + 64 : 65 * i + 65]
                            )
                            ao = sb1.tile([128, DK], F32, tag="ao")
                            nc.scalar.activation(
                                ao[:],
                                av[:, 65 * i : 65 * i + DK],
                                AF.Copy,
                                scale=rz[:],
                            )
                            ps_aoT = sps.tile(
                                [128, 512], F32, tag="sp", name=f"paoT_{qt5}_{i}_{h}"
                            )
                            nc.tensor.transpose(
                                ps_aoT[0:DK, 0:128], ao[:], ident_f32[:]
                            )
                            nc.vector.tensor_copy(
                                aoT[po : po + DK, q0 : q0 + 128], ps_aoT[0:DK, 0:128]
                            )

                        ps_o = sps.tile(
                            [128, 512], F32, tag="sp", name=f"po_{qt5}_{i}"
                        )
                        nc.tensor.matmul(
                            ps_o[:],
                            aoT[:, q0 : q0 + 128],
                            woT_sb[:],
                            start=True,
                            stop=True,
                        )
                        o_sb = sb1.tile([128, D], F32, tag="osb")
                        nc.vector.tensor_copy(o_sb[:], ps_o[:])
                        nc.sync.dma_start(
                            out=out_d.ap()[q0 : q0 + 128, :], in_=o_sb[:]
                        )

    nc.compile()
    return nc


def _emit_av(nc, probsT, sc, avs, Vsb):
    for h in range(2):
        av = avs[h]
        for i in range(4):
            nc.tensor.matmul(
                av[:, 65 * i : 65 * i + 65],
                probsT[:, 512 * h + 128 * i : 512 * h + 128 * i + 128],
                Vsb[:, 130 * sc + 65 * h : 130 * sc + 65 * (h + 1)],
                start=(sc == 0 and i == 0),
                stop=(sc == NSC - 1 and i == 3),
                skip_group_check=True,
            )


def _core_inputs(inputs, core):
    PROJDT, _, _, _ = _dtypes()
    pdt = _np_dt(PROJDT)

    x = np.asarray(inputs["x"], dtype=np.float32)
    pos_emb = np.asarray(inputs["pos_emb"], dtype=np.float32)
    W_qkv = np.asarray(inputs["W_qkv"], dtype=np.float32)
    W_pos = np.asarray(inputs["W_pos"], dtype=np.float32)
    W_out = np.asarray(inputs["W_out"], dtype=np.float32)
    u = np.asarray(inputs["pos_bias_u"], dtype=np.float32)
    v = np.asarray(inputs["pos_bias_v"], dtype=np.float32)

    b = core // 4
    h0 = 2 * (core % 4)
    r0 = h0 * DK

    def swz(wT):  # [512, 128] -> [128, 512] laid out as (p, (k, m))
        return np.ascontiguousarray(
            wT.reshape(4, 128, 128).transpose(1, 0, 2).reshape(128, 512)
        )

    return {
        "xT": np.ascontiguousarray(x[b].T).astype(pdt),
        "posTe": np.ascontiguousarray(pos_emb[0].T).astype(pdt),
        "wqT": swz((W_qkv[r0 : r0 + 128, :].T * SCALE).astype(pdt)),
        "wkT": swz(W_qkv[D + r0 : D + r0 + 128, :].T.astype(pdt)),
        "wvT": swz(W_qkv[2 * D + r0 : 2 * D + r0 + 128, :].T.astype(pdt)),
        "wposT": swz(W_pos[r0 : r0 + 128, :].T.astype(pdt)),
        "woT": np.ascontiguousarray(W_out[:, r0 : r0 + 128].T),
        "bias_u": (np.concatenate([u[h0], u[h0 + 1]]).reshape(128, 1) * SCALE),
        "bias_v": (np.concatenate([v[h0], v[h0 + 1]]).reshape(128, 1) * SCALE),
    }


def kernel(**inputs) -> np.ndarray:
    global _NC, _LAST_RESULTS
    from concourse.bass_utils import run_bass_kernel_spmd

    if _NC is None:
        _NC = _build_nc()

    in_maps = [_core_inputs(inputs, c) for c in range(NCORES)]
    trace = os.environ.get("KERNEL_TRACE", "0") == "1"
    res = run_bass_kernel_spmd(
        _NC,
        in_maps,
        core_ids=list(range(NCORES)),
        trace=trace,
        trace_cores=[0] if trace else None,
    )
    _LAST_RESULTS = res

    out = np.zeros((2, T, D), dtype=np.float32)
    for c in range(NCORES):
        out[c // 4] += res.results[c]["outp"]
    return out


# revision 37
# speedup vs baseline: 1.0034x; 1.0034x over previous
"""ConformerAttention (Transformer-XL relative attention) on 8 TRN2 NeuronCores.

Sharding: batch*heads across cores. Core c handles batch b = c//4 and the head
pair (h0, h1) = (2*(c%4), 2*(c%4)+1). All projections, the rel-shift bias, the
softmax and the attention are computed per (b, head-pair) on one core; the
output projection is computed per-core against that pair's W_out columns and
the 4 partial [T, D] outputs per batch are summed on the host (the gather).

v2 dataflow (transposed scores, PE-side bias merge):
  - Scores are computed directly TRANSPOSED: qkT[s, q] = (K^T)^T_chunk @ Qu
    per (s-128-chunk, q-512-tile), FD=512, K=64.  The two heads' matmuls are
    emitted back-to-back on row groups (0,0)/(64,0) into the two banks of one
    [128, 1024] PSUM tile, so they run CONCURRENTLY on the PE array halves.
  - The rel-shift bias band bd_al[q, s] (built per q-128-tile via the skewed
    strip + SBUF->SBUF DMA, as before) is merged by a plain matmul
    out += bd_al^T = lhsT(bd_al) @ I  with start=False, accumulating straight
    into the qkT PSUM bank.  No DVE add, no score staging, no extra casts.
  - One exp() per s-chunk drains the combined [128, 1024] PSUM (both heads)
    into probsT[s, (h,q)] bf16, which feeds attn@[V|1] directly (row sum Z
    rides as the 65th rhs column, as before).
  - av is emitted one s-chunk behind exp so the PE never waits on ACT.
"""

import os

import numpy as np

T = 2048
D = 512
NH = 8
DK = 64
P = 2 * T - 1
NCORES = 8
NQT = T // 128  # 16 q-subtiles of 128 rows
NSC = T // 128  # 16 s-chunks of 128 cols
SW = T + 127  # 2175 band strip width per q-subtile
SCALE = np.float32(1.0 / np.sqrt(DK))

_NC = None
_LAST_RESULTS = None


def _dt(name, default):
    import concourse.mybir as mybir

    return {"f32": mybir.dt.float32, "bf16": mybir.dt.bfloat16}[
        os.environ.get(name, default)
    ]


def _dtypes():
    return (
        _dt("KERNEL_PROJDT", "bf16"),
        _dt("KERNEL_MMDT", "bf16"),
        _dt("KERNEL_BANDDT", "bf16"),
        _dt("KERNEL_SDT", "bf16"),
    )


def _np_dt(dt):
    import concourse.mybir as mybir

    return mybir.dt.np(dt)


def _build_nc():
    import concourse.bacc as bacc
    import concourse.bass as bass
    import concourse.mybir as mybir
    import concourse.tile as tile
    from concourse import masks

    F32 = mybir.dt.float32
    PROJDT, MMDT, BANDDT, SDT = _dtypes()
    AF = mybir.ActivationFunctionType

    nc = bacc.Bacc("TRN2", target_bir_lowering=False, debug=False)

    xT_d = nc.dram_tensor("xT", [D, T], PROJDT, kind="ExternalInput")
    posTe_d = nc.dram_tensor("posTe", [D, P], PROJDT, kind="ExternalInput")
    wqT_d = nc.dram_tensor("wqT", [128, 512], PROJDT, kind="ExternalInput")
    wkT_d = nc.dram_tensor("wkT", [128, 512], PROJDT, kind="ExternalInput")
    wvT_d = nc.dram_tensor("wvT", [128, 512], PROJDT, kind="ExternalInput")
    wposT_d = nc.dram_tensor("wposT", [128, 512], PROJDT, kind="ExternalInput")
    woT_d = nc.dram_tensor("woT", [128, D], F32, kind="ExternalInput")
    bu_d = nc.dram_tensor("bias_u", [128, 1], F32, kind="ExternalInput")
    bv_d = nc.dram_tensor("bias_v", [128, 1], F32, kind="ExternalInput")
    out_d = nc.dram_tensor("outp", [T, D], F32, kind="ExternalOutput")

    with tile.TileContext(nc) as tc:
        with (
            tc.tile_pool(name="const", bufs=1) as constp,
            tc.tile_pool(name="pers", bufs=1) as pers,
            # PSUM budget (8 banks): qkT 2x[128,1024] = 4, sps 2x[128,512] = 2,
            # av 2x[128,260] = 2
            tc.tile_pool(name="qkps", bufs=2, space="PSUM") as qkp,
            tc.tile_pool(name="sps", bufs=2, space="PSUM") as sps,
            tc.tile_pool(name="avps", bufs=1, space="PSUM") as avp,
            tc.tile_pool(name="sb1", bufs=2) as sb1,
        ):
            ident_f32 = constp.tile([128, 128], F32)
            masks.make_identity(nc, ident_f32[:])
            ident_b = constp.tile([128, 128], BANDDT)
            masks.make_identity(nc, ident_b[:])

            bu_sb = constp.tile([128, 1], F32)
            nc.gpsimd.dma_start(out=bu_sb[:], in_=bu_d.ap())
            bv_sb = constp.tile([128, 1], F32)
            nc.gpsimd.dma_start(out=bv_sb[:], in_=bv_d.ap())
            woT_sb = constp.tile([128, D], F32)
            nc.gpsimd.dma_start(out=woT_sb[:], in_=woT_d.ap())

            QuT = pers.tile([128, T], MMDT)
            QvT = pers.tile([128, T], MMDT)
            KT = pers.tile([128, T], MMDT)
            posT = pers.tile([128, P], MMDT)
            Vsb = pers.tile([128, NSC * 130], SDT)
            aoT = pers.tile([128, T], F32)

            # ones columns for the fused row-sum (col 64 of each rhs slice)
            v3 = Vsb[:].rearrange("p (j c) -> p j c", c=130)
            nc.vector.memset(v3[:, :, 64:65], 1.0)
            nc.vector.memset(v3[:, :, 129:130], 1.0)

            # ---------------- phase 0: projections ----------------
            with (
                tc.tile_pool(name="posp", bufs=1) as posp,
                tc.tile_pool(name="ph0", bufs=1) as ph0p,
            ):
                xT_sb = []
                for kc in range(4):
                    t = ph0p.tile([128, T], PROJDT, tag=f"xT{kc}")
                    nc.sync.dma_start(
                        out=t[:], in_=xT_d.ap()[128 * kc : 128 * (kc + 1), :]
                    )
                    xT_sb.append(t)

                def load_wT(dr, pool, nm):
                    t = pool.tile([128, 512], PROJDT, tag=nm, name=nm)
                    nc.sync.dma_start(out=t[:], in_=dr.ap())
                    return [t[:, 128 * kc : 128 * (kc + 1)] for kc in range(4)]

                wq_sb = load_wT(wqT_d, ph0p, "wq")
                wk_sb = load_wT(wkT_d, ph0p, "wk")
                wv_sb = load_wT(wvT_d, ph0p, "wv")
                wp_sb = load_wT(wposT_d, posp, "wp")

                pos_stage = {}

                def load_pos_chunk(n8):
                    w0 = 512 * n8
                    ncols = min(512, P - w0)
                    ts = []
                    for kc in range(4):
                        t = posp.tile(
                            [128, 512], PROJDT, tag=f"pe{kc}", bufs=3,
                            name=f"pe{kc}_{n8}",
                        )
                        nc.gpsimd.dma_start(
                            out=t[:, :ncols],
                            in_=posTe_d.ap()[
                                128 * kc : 128 * (kc + 1), w0 : w0 + ncols
                            ],
                        )
                        ts.append(t)
                    pos_stage[n8] = ts

                def mm_pos_chunk(n8):
                    w0 = 512 * n8
                    ncols = min(512, P - w0)
                    pe_sb = pos_stage.pop(n8)
                    ps = sps.tile([128, 512], F32, tag="sp", name=f"p0p_{n8}")
                    for kc in range(4):
                        nc.tensor.matmul(
                            ps[:, :ncols],
                            wp_sb[kc][:],
                            pe_sb[kc][:, :ncols],
                            start=(kc == 0),
                            stop=(kc == 3),
                        )
                    nc.vector.tensor_copy(posT[:, w0 : w0 + ncols], ps[:, :ncols])

                # prefetch the first pos chunks; their loads overlap Q/K/V
                for n8 in (7, 6, 5):
                    load_pos_chunk(n8)

                # Q^T and K^T (both heads stacked on partitions)
                for w_sb, drains in (
                    (
                        wq_sb,
                        lambda ps, sl: (
                            nc.scalar.activation(
                                QuT[:, sl], ps[:], AF.Identity, bias=bu_sb[:]
                            ),
                            nc.scalar.activation(
                                QvT[:, sl], ps[:], AF.Identity, bias=bv_sb[:]
                            ),
                        ),
                    ),
                    (
                        wk_sb,
                        lambda ps, sl: nc.vector.tensor_copy(KT[:, sl], ps[:]),
                    ),
                ):
                    for n4 in range(4):
                        sl = slice(512 * n4, 512 * (n4 + 1))
                        ps = sps.tile([128, 512], F32, tag="sp", name=f"p0_{n4}")
                        for kc in range(4):
                            nc.tensor.matmul(
                                ps[:],
                                w_sb[kc][:],
                                xT_sb[kc][:, sl],
                                start=(kc == 0),
                                stop=(kc == 3),
                            )
                        drains(ps, sl)

                # V (both heads)
                for tt in range(NSC):
                    ps = sps.tile([128, 512], F32, tag="sp", name=f"pv_{tt}")
                    for kc in range(4):
                        nc.tensor.matmul(
                            ps[:, 0:128],
                            xT_sb[kc][:, 128 * tt : 128 * (tt + 1)],
                            wv_sb[kc][:],
                            start=(kc == 0),
                            stop=(kc == 3),
                        )
                    nc.vector.tensor_copy(
                        Vsb[:, 130 * tt : 130 * tt + 64], ps[:, 0:64]
                    )
                    nc.vector.tensor_copy(
                        Vsb[:, 130 * tt + 65 : 130 * tt + 129], ps[:, 64:128]
                    )

                # ---------------- phase 1: attention ----------------
                # pos chunks projected lazily: qt512=0 needs chunks 3..7,
                # each later qt512 needs one lower chunk.
                mm_pos_chunk(7)
                load_pos_chunk(4)
                mm_pos_chunk(6)
                load_pos_chunk(3)
                for n8 in (5, 4, 3):
                    mm_pos_chunk(n8)

                strip_cur = {}

                def emit_strip_op(qt5, i, h, sc5, out):
                    """One 512-col band sub-chunk matmul + drain for
                    (q-subtile i, head h) of tile qt5.  After the last
                    sub-chunk, issues the skewed rel-shift DMA into out."""
                    q0 = 512 * qt5 + 128 * i
                    qbar = (T - 1) - q0 - 127
                    w = 512 if sc5 < 4 else SW - 4 * 512
                    po = 64 * h
                    key = (qt5, i, h)
                    if key not in strip_cur:
                        strip_cur[key] = sb1.tile(
                            [128, SW], BANDDT, tag=f"strip{i}_{h}", bufs=2,
                            name=f"strip{i}_{h}_{qt5}",
                        )
                    strip = strip_cur[key]
                    ps_bd = sps.tile(
                        [128, 512], F32, tag="sp",
                        name=f"bd_{qt5}_{i}_{h}_{sc5}",
                    )
                    nc.tensor.matmul(
                        ps_bd[:, :w],
                        QvT[po : po + 64, q0 : q0 + 128],
                        posT[
                            po : po + 64,
                            qbar + 512 * sc5 : qbar + 512 * sc5 + w,
                        ],
                        start=True,
                        stop=True,
                    )
                    # big drains on DVE (ACT's FIFO must stay clear for exp);
                    # only the cheap 127-col tails go to ACT
                    if sc5 == 4:
                        nc.scalar.copy(
                            strip[:, 512 * sc5 : 512 * sc5 + w], ps_bd[:, :w]
                        )
                    else:
                        nc.vector.tensor_copy(
                            strip[:, 512 * sc5 : 512 * sc5 + w], ps_bd[:, :w]
                        )
                    if sc5 == 4:
                        t = sb1.tile(
                            [128, T], BANDDT, tag=f"bdal{i}_{h}", bufs=2,
                            name=f"bdal{i}_{h}_{qt5}",
                        )
                        eng = nc.sync if h == 0 else nc.scalar
                        eng.dma_start(
                            out=t[:],
                            in_=bass.AP(
                                strip.tensor, 127, [[SW - 1, 128], [1, T]]
                            ),
                        )
                        out[(i, h)] = t

                # pipeline fill: strips + skew for tile 0
                bd_al = {}
                for i in range(4):
                    for h in range(2):
                        for sc5 in range(5):
                            emit_strip_op(0, i, h, sc5, bd_al)

                # next-tile band work, spread through the s-chunk loop:
                # per (i, h) the sub-chunks run (1,2,3,0,4) so the ones
                # touching the freshly-projected low pos chunk come later
                STRIP_TASKS = [
                    (i, h, s5)
                    for i in range(4)
                    for s5 in (1, 2, 3, 0, 4)
                    for h in range(2)
                ]

                for qt5 in range(4):
                    q0t = 512 * qt5  # q-512 tile base
                    bd_al_next = {}
                    sti = [0]

                    def pop_strip():
                        # emit BOTH heads of one (i, sc5) band sub-chunk
                        # back-to-back: alternating row groups let the 2nd
                        # LDWEIGHTS pull ahead under the 1st matmul
                        for _ in range(2):
                            if qt5 < 3 and sti[0] < len(STRIP_TASKS):
                                i_, h_, s5_ = STRIP_TASKS[sti[0]]
                                sti[0] += 1
                                emit_strip_op(qt5 + 1, i_, h_, s5_, bd_al_next)

                    # --- s-chunk loop ---
                    # [128, 512] = exactly one PSUM bank per head; av slices at
                    # 65-col offsets share the bank. Only the FIRST matmul into
                    # the bank may use start=True: start marks the whole
                    # 2 KiB zero-region pending, so a second start would wipe
                    # sibling slices' sc=0 contribution.
                    avs = [
                        avp.tile([128, 512], F32, tag=f"av{h}", name=f"av{h}_{qt5}")
                        for h in range(2)
                    ]
                    prev = None  # (probsT tile, sc) pending av
                    for sc in range(NSC):
                        s0 = 128 * sc
                        if sc == 0 and qt5 < 3:
                            load_pos_chunk(2 - qt5)
                        if sc == 1 and qt5 < 3:
                            mm_pos_chunk(2 - qt5)
                        ps = qkp.tile([128, 1024], F32, tag="qkT")
                        # content scores, both heads back-to-back on row
                        # groups (0,0)/(64,0) -> the two banks of ps
                        for h in range(2):
                            po = 64 * h
                            nc.tensor.matmul(
                                ps[:, 512 * h : 512 * h + 512],
                                KT[po : po + 64, s0 : s0 + 128],
                                QuT[po : po + 64, q0t : q0t + 512],
                                start=True,
                                stop=False,
                            )
                        if sc >= 1:
                            pop_strip()
                        # merge rel-shift bias: ps += bd_al^T via identity
                        for h in range(2):
                            for i in range(4):
                                nc.tensor.matmul(
                                    ps[:, 512 * h + 128 * i : 512 * h + 128 * i + 128],
                                    bd_al[(i, h)][:, s0 : s0 + 128],
                                    ident_b[:],
                                    start=False,
                                    stop=(i == 3),
                                )
                        probsT = sb1.tile([128, 1024], SDT, tag="probsT", bufs=2)
                        # per-head exp: h0's drain starts while h1's bias
                        # merges are still on the PE
                        for h in range(2):
                            nc.scalar.activation(
                                probsT[:, 512 * h : 512 * h + 512],
                                ps[:, 512 * h : 512 * h + 512],
                                AF.Exp,
                            )

                        if prev is not None:
                            _emit_av(nc, prev[0], prev[1], avs, Vsb)
                        prev = (probsT, sc)
                        if sc >= 1:
                            pop_strip()
                    _emit_av(nc, prev[0], prev[1], avs, Vsb)
                    bd_al = bd_al_next

                    # --- epilogue: normalize, transpose, project ---
                    for i in range(4):
                        q0 = q0t + 128 * i
                        for h in range(2):
                            po = 64 * h
                            av = avs[h]
                            rz = sb1.tile([128, 1], F32, tag="rz")
                            nc.vector.reciprocal(
                                rz[:], av[:, 65 * i + 64 : 65 * i + 65]
                            )
                            ao = sb1.tile([128, DK], F32, tag="ao")
                            nc.scalar.activation(
                                ao[:],
                                av[:, 65 * i : 65 * i + DK],
                                AF.Copy,
                                scale=rz[:],
                            )
                            ps_aoT = sps.tile(
                                [128, 512], F32, tag="sp", name=f"paoT_{qt5}_{i}_{h}"
                            )
                            nc.tensor.transpose(
                                ps_aoT[0:DK, 0:128], ao[:], ident_f32[:]
                            )
                            nc.vector.tensor_copy(
                                aoT[po : po + DK, q0 : q0 + 128], ps_aoT[0:DK, 0:128]
                            )

                        ps_o = sps.tile(
                            [128, 512], F32, tag="sp", name=f"po_{qt5}_{i}"
                        )
                        nc.tensor.matmul(
                            ps_o[:],
                            aoT[:, q0 : q0 + 128],
                            woT_sb[:],
                            start=True,
                            stop=True,
                        )
                        o_sb = sb1.tile([128, D], F32, tag="osb")
                        nc.vector.tensor_copy(o_sb[:], ps_o[:])
                        nc.sync.dma_start(
                            out=out_d.ap()[q0 : q0 + 128, :], in_=o_sb[:]
                        )

    nc.compile()
    return nc


def _emit_av(nc, probsT, sc, avs, Vsb):
    for h in range(2):
        av = avs[h]
        for i in range(4):
            nc.tensor.matmul(
                av[:, 65 * i : 65 * i + 65],
                probsT[:, 512 * h + 128 * i : 512 * h + 128 * i + 128],
                Vsb[:, 130 * sc + 65 * h : 130 * sc + 65 * (h + 1)],
                start=(sc == 0 and i == 0),
                stop=(sc == NSC - 1 and i == 3),
                skip_group_check=True,
            )


def _core_inputs(inputs, core):
    PROJDT, _, _, _ = _dtypes()
    pdt = _np_dt(PROJDT)

    x = np.asarray(inputs["x"], dtype=np.float32)
    pos_emb = np.asarray(inputs["pos_emb"], dtype=np.float32)
    W_qkv = np.asarray(inputs["W_qkv"], dtype=np.float32)
    W_pos = np.asarray(inputs["W_pos"], dtype=np.float32)
    W_out = np.asarray(inputs["W_out"], dtype=np.float32)
    u = np.asarray(inputs["pos_bias_u"], dtype=np.float32)
    v = np.asarray(inputs["pos_bias_v"], dtype=np.float32)

    b = core // 4
    h0 = 2 * (core % 4)
    r0 = h0 * DK

    def swz(wT):  # [512, 128] -> [128, 512] laid out as (p, (k, m))
        return np.ascontiguousarray(
            wT.reshape(4, 128, 128).transpose(1, 0, 2).reshape(128, 512)
        )

    return {
        "xT": np.ascontiguousarray(x[b].T).astype(pdt),
        "posTe": np.ascontiguousarray(pos_emb[0].T).astype(pdt),
        "wqT": swz((W_qkv[r0 : r0 + 128, :].T * SCALE).astype(pdt)),
        "wkT": swz(W_qkv[D + r0 : D + r0 + 128, :].T.astype(pdt)),
        "wvT": swz(W_qkv[2 * D + r0 : 2 * D + r0 + 128, :].T.astype(pdt)),
        "wposT": swz(W_pos[r0 : r0 + 128, :].T.astype(pdt)),
        "woT": np.ascontiguousarray(W_out[:, r0 : r0 + 128].T),
        "bias_u": (np.concatenate([u[h0], u[h0 + 1]]).reshape(128, 1) * SCALE),
        "bias_v": (np.concatenate([v[h0], v[h0 + 1]]).reshape(128, 1) * SCALE),
    }


def kernel(**inputs) -> np.ndarray:
    global _NC, _LAST_RESULTS
    from concourse.bass_utils import run_bass_kernel_spmd

    if _NC is None:
        _NC = _build_nc()

    in_maps = [_core_inputs(inputs, c) for c in range(NCORES)]
    trace = os.environ.get("KERNEL_TRACE", "0") == "1"
    res = run_bass_kernel_spmd(
        _NC,
        in_maps,
        core_ids=list(range(NCORES)),
        trace=trace,
        trace_cores=[0] if trace else None,
    )
    _LAST_RESULTS = res

    out = np.zeros((2, T, D), dtype=np.float32)
    for c in range(NCORES):
        out[c // 4] += res.results[c]["outp"]
    return out
